# revision 54
# baseline (speedup 1.0000x reference)
"""Trainium2 Bass kernel for 2-layer GAT (nn_FAGAT) over 8 NeuronCores.

v2 design (aggregate-then-project, fp8-resident one-hot scatter):
  - dst blocks (128 nodes) are dealt round-robin by edge count across the 8
    cores to equalize per-slot chunk profiles (SPMD: one program, per-core
    data).  Node tables live in *dealt position* order so layer-1 and layer-2
    gathers share one chunk structure.
  - Layer 1 exploits linearity: out1[d] = W1.T (sum_e w_e x_e) / den, so the
    per-edge work happens on 27-dim x (xq = x (x) per-head w, one [128,132]
    matmul per 128-edge chunk against the resident one-hot S), and the dense
    W1 projection runs once per dst block.
  - One-hot S matrices (edge-major) are built once per chunk by DVE is_equal
    in fp8e4 and stay SBUF-resident for both layers; ST (dst-major, for the
    s_dst broadcast matmul) is PE-transposed from S once.  Matmuls mix fp8
    lhsT with bf16 moving operands.
  - Attention: s_src is host-precomputed into the gather row; s_dst expands
    per edge via tiny ST@sdst matmuls; leaky-relu and exp run on the scalar
    engine batched per 16-chunk gather window.  All of {Copy,Exp,Lrelu,Relu}
    live in one activation table set; sigmoid is deferred to a single call at
    the end to avoid table swaps.
  - Tables are bf16 (256B rows for x/s_src1, 512B rows for the layer-2
    h2/s_src2/s_dst2 table); int16 gather indices use lo/hi split streams at
    position 32768.
  - Softmax without running max: logits are bounded for these inputs, exp()
    is safe, alpha = e/(sum+eps) matches the reference up to ~1e-16.
"""
import os
os.environ.setdefault("NEURON_SCRATCHPAD_PAGE_SIZE", "64")
import sys
if "/opt/trn_rl_repo" not in sys.path:
    sys.path.insert(0, "/opt/trn_rl_repo")

from dataclasses import dataclass, field
import numpy as np
import ml_dtypes
NP_BF16 = np.dtype(ml_dtypes.bfloat16)
NP_F8 = np.dtype(ml_dtypes.float8_e4m3fn)

import concourse.bass as bass
import concourse.mybir as mybir
from concourse import bacc, tile
from concourse.bass_utils import run_bass_kernel_spmd

F32 = mybir.dt.float32
BF16 = mybir.dt.bfloat16
F8 = mybir.dt.float8e4
I16 = mybir.dt.int16
AF = mybir.ActivationFunctionType
OP = mybir.AluOpType

NEG = 0.2
EPS = 1e-16


@dataclass
class Cfg:
    N: int = 50000
    NC: int = 8
    SPLIT: int = 32768
    KIN: int = 27
    H1: int = 4
    D1: int = 64
    H2: int = 2
    D2: int = 64
    WCH: int = 16
    XROW: int = 128            # x table row (bf16)
    CROW: int = 256            # layer-2 table row (bf16)
    use_f8: bool = True
    timing_single_core: bool = False

    @property
    def NBLK_G(self):
        return (self.N + 127) // 128          # 391 global blocks

    @property
    def NBLK(self):
        return (self.NBLK_G + self.NC - 1) // self.NC   # 49 slots per core

    @property
    def NPOS(self):
        return self.NC * self.NBLK * 128      # 50176 table rows

    @property
    def F1(self):
        return self.H1 * self.D1

    @property
    def F2(self):
        return self.H2 * self.D2


@dataclass
class Structure:
    deal: list = None            # deal[c] = list of global block ids
    chunks: list = field(default_factory=list)
    win_chunks: dict = field(default_factory=dict)
    cores: list = field(default_factory=list)
    NLO: int = 0
    NHI: int = 0
    NCH: int = 0
    pos: np.ndarray = None
    add_b1: bool = False
    add_b2: bool = False


def _wrap_idx(a, nch):
    w = a.astype(np.int16).reshape(nch * 8, 16).T
    return np.tile(w, (8, 1)).copy()


def prep_edges(cfg: Cfg, src, dst):
    src = np.asarray(src, dtype=np.int64)
    dst = np.asarray(dst, dtype=np.int64)
    NBLK_G, NBLK, NC = cfg.NBLK_G, cfg.NBLK, cfg.NC

    gb = dst // 128
    cnt = np.bincount(gb, minlength=NBLK_G)
    order = np.argsort(-cnt, kind="stable")
    deal = [[] for _ in range(NC)]
    for i, b in enumerate(order):
        deal[i % NC].append(int(b))

    # node -> table position
    core_of = np.zeros(NBLK_G, np.int64)
    slot_of = np.zeros(NBLK_G, np.int64)
    for c in range(NC):
        for k, b in enumerate(deal[c]):
            core_of[b] = c
            slot_of[b] = k
    nodes = np.arange(cfg.N, dtype=np.int64)
    pos = core_of[nodes // 128] * (NBLK * 128) + slot_of[nodes // 128] * 128 \
        + (nodes % 128)

    spos = pos[src]
    dloc_all = dst % 128

    # per (core, slot): edge lists split lo/hi by src position
    per = {}
    for c in range(NC):
        for k, b in enumerate(deal[c]):
            m = gb == b
            sp, dl = spos[m], dloc_all[m]
            lo = sp < cfg.SPLIT
            per[(c, k)] = ((sp[lo], dl[lo]), (sp[~lo] - cfg.SPLIT, dl[~lo]))

    nlo = np.zeros(NBLK, int)
    nhi = np.zeros(NBLK, int)
    for (c, k), ((ls, _), (hs, _)) in per.items():
        nlo[k] = max(nlo[k], -(-len(ls) // 128))
        nhi[k] = max(nhi[k], -(-len(hs) // 128))
    nlo = np.maximum(nlo, 1)

    st = Structure(deal=deal, pos=pos)
    slot_ctr = {"lo": 0, "hi": 0}
    for k in range(NBLK):
        tot = int(nlo[k] + nhi[k])
        j = 0
        for kind, nch in (("lo", int(nlo[k])), ("hi", int(nhi[k]))):
            for _ in range(nch):
                ks = slot_ctr[kind]
                st.chunks.append((kind, k, j == 0, j == tot - 1, ks))
                w, wi = divmod(ks, cfg.WCH)
                st.win_chunks.setdefault((kind, w), []).append((wi, k))
                slot_ctr[kind] += 1
                j += 1
    st.NLO, st.NHI = slot_ctr["lo"], slot_ctr["hi"]
    st.NCH = st.NLO + st.NHI

    for c in range(NC):
        idx = {"lo": np.zeros(st.NLO * 128, np.int32),
               "hi": np.zeros(st.NHI * 128, np.int32)}
        dlc = {"lo": np.full(st.NLO * 128, -1.0, np.float32),
               "hi": np.full(st.NHI * 128, -1.0, np.float32)}
        ofs = {"lo": 0, "hi": 0}
        for k in range(NBLK):
            for kind, nch in (("lo", int(nlo[k])), ("hi", int(nhi[k]))):
                if (c, k) in per:
                    arr_i, arr_d = per[(c, k)][0 if kind == "lo" else 1]
                    o = ofs[kind] * 128
                    idx[kind][o:o + len(arr_i)] = arr_i
                    dlc[kind][o:o + len(arr_d)] = arr_d
                ofs[kind] += nch
        core = dict(
            idx_lo=_wrap_idx(idx["lo"], st.NLO),
            idx_hi=_wrap_idx(idx["hi"], st.NHI),
        )
        # one-hot S (edge-major) / ST (dst-major) fp8 tables
        for kind, ncnt in (("lo", st.NLO), ("hi", st.NHI)):
            dl = dlc[kind].reshape(ncnt, 128)          # [slot, edge p]
            Sm = np.zeros((128, ncnt, 128), NP_F8)
            Tm = np.zeros((128, ncnt, 128), NP_F8)
            sl, pe = np.nonzero(dl >= 0)
            dv = dl[sl, pe].astype(np.int64)
            Sm[pe, sl, dv] = 1.0
            Tm[dv, sl, pe] = 1.0
            core[f"S_{kind}"] = Sm
            core[f"T_{kind}"] = Tm
        st.cores.append(core)
    return st


def host_inputs(cfg: Cfg, st: Structure, inputs):
    bf = NP_BF16
    x = np.asarray(inputs["x"], np.float32)
    W1 = np.asarray(inputs["W1"], np.float32)
    a_src1 = np.asarray(inputs["a_src1"], np.float32)
    a_dst1 = np.asarray(inputs["a_dst1"], np.float32)
    W2 = np.asarray(inputs["W2"], np.float32)
    a_src2 = np.asarray(inputs["a_src2"], np.float32)
    a_dst2 = np.asarray(inputs["a_dst2"], np.float32)

    H1, D1, H2, D2, KIN = cfg.H1, cfg.D1, cfg.H2, cfg.D2, cfg.KIN
    As1 = np.stack([W1[:, h * D1:(h + 1) * D1] @ a_src1[h] for h in range(H1)], 1)
    Ad1 = np.stack([W1[:, h * D1:(h + 1) * D1] @ a_dst1[h] for h in range(H1)], 1)
    s_src1 = x @ As1      # [N, H1]
    s_dst1 = x @ Ad1

    # x table in dealt-position order: [x(27) | 0 | s_src1(4) | 0...]
    x_tab = np.zeros((cfg.NPOS, cfg.XROW), bf)
    x_tab[st.pos, :KIN] = x.astype(bf)
    x_tab[st.pos, 28:32] = s_src1.astype(bf)

    # Wbig [128, 256]: block-diagonal W1 per head (rows h*32+k, k<27)
    Wbig = np.zeros((128, cfg.F1), np.float32)
    for h in range(H1):
        Wbig[h * 32:h * 32 + KIN, h * D1:(h + 1) * D1] = W1[:, h * D1:(h + 1) * D1]

    # W2ext [256, 132] -> [128, 2, 132]
    W2e = np.concatenate([W2,
                          np.stack([W2[:, h * D2:(h + 1) * D2] @ a_src2[h]
                                    for h in range(H2)], 1),
                          np.stack([W2[:, h * D2:(h + 1) * D2] @ a_dst2[h]
                                    for h in range(H2)], 1)], axis=1)  # [256,132]
    W2e = np.ascontiguousarray(
        W2e.reshape(2, 128, 132).transpose(1, 0, 2))

    iota = np.tile(np.arange(128, dtype=np.float32), (128, 1))
    ident = np.eye(128, dtype=np.float32)

    shared = dict(
        x_tab=x_tab,
        WBIG=Wbig.astype(bf),
        W2E=W2e.astype(bf),
        WFC=np.asarray(inputs["Wfc"], np.float32).reshape(128, 1).astype(bf),
        IOTA=iota.astype(bf),
        IDENTB=ident.astype(bf),
        IDENT8=ident.astype(NP_F8),
        B1ROW=np.tile(np.asarray(inputs["b1"], np.float32)[None, :], (128, 1)),
        B2ROW=np.tile(np.asarray(inputs["b2"], np.float32)[None, :], (128, 1)),
    )

    in_maps = []
    for c in range(cfg.NC):
        m = dict(shared)
        m.update(st.cores[c])
        # s_dst1 per slot: [128, NBLK, H1] bf16
        sd = np.zeros((128, cfg.NBLK, H1), np.float32)
        for k, b in enumerate(st.deal[c]):
            rows = min(128, cfg.N - b * 128)
            sd[:rows, k, :] = s_dst1[b * 128:b * 128 + rows]
        m["SDST1"] = sd.astype(bf)
        in_maps.append(m)
    return in_maps


# --------------------------------------------------------------------------
#  device program
# --------------------------------------------------------------------------

def emit_gat(tc, outs, ins, cfg: Cfg, st: Structure):
    nc = tc.nc
    NBLK, WCH, H1, H2, F1, F2 = cfg.NBLK, cfg.WCH, cfg.H1, cfg.H2, cfg.F1, cfg.F2
    y = outs["y"]
    nslots = {"lo": st.NLO, "hi": st.NHI}

    cc_in = nc.dram_tensor("cc_in", [NBLK * 128, cfg.CROW], BF16,
                           kind="Internal").ap()
    cc_out = nc.dram_tensor("cc_out", [cfg.NPOS, cfg.CROW], BF16,
                            kind="Internal", addr_space="Shared").ap()

    with (
        tc.tile_pool(name="const", bufs=1) as constp,
        tc.tile_pool(name="resid", bufs=1) as residp,
    ):
        def cload(name, dtype=None):
            src = ins[name]
            t = constp.tile(list(src.shape), dtype or src.dtype,
                            tag=name, name=name)
            nc.sync.dma_start(t[:], src)
            return t

        IDENTB = cload("IDENTB")
        WBIG = cload("WBIG")
        W2E = cload("W2E")
        WFC = cload("WFC")
        SDST1 = cload("SDST1")
        IXLO = cload("idx_lo")
        IXHI = cload("idx_hi")
        B1R = cload("B1ROW") if st.add_b1 else None
        B2R = cload("B2ROW") if st.add_b2 else None
        idx_t = {"lo": IXLO, "hi": IXHI}

        # resident one-hot matrices (fp8), built during layer 1
        S_lo = residp.tile([128, st.NLO, 128], F8, name="S_lo")
        S_hi = residp.tile([128, st.NHI, 128], F8, name="S_hi")
        S_t = {"lo": S_lo, "hi": S_hi}
        S_dram = {"lo": ins["S_lo"], "hi": ins["S_hi"]}
        T_dram = {"lo": ins["T_lo"], "hi": ins["T_hi"]}

        z_all = residp.tile([128, NBLK], F32, name="z_all")

        def win_setup(layer, gpool, xwpool, swps, stps, tab_lo, tab_hi, elem,
                      scol, H, xww, sdst_tile, windows, build_S):
            """Fetch gather window + attention weights; returns tiles."""
            def get(kind, w):
                key = (kind, w)
                if key in windows:
                    return windows[key]
                n = min(WCH, nslots[kind] - w * WCH)
                k0 = w * WCH
                gt = gpool.tile([128, WCH, elem], BF16, tag=f"g{kind}",
                                name=f"gt{layer}")
                tab = tab_lo if kind == "lo" else tab_hi
                for g0 in range(0, n, 8):
                    gn = min(8, n - g0)
                    nidx = gn * 128
                    nc.gpsimd.dma_gather(
                        gt[:, g0:g0 + gn, :], tab,
                        idx_t[kind][:, (k0 + g0) * 8:(k0 + g0 + gn) * 8],
                        nidx, nidx, elem)
                stw = xwpool.tile([128, WCH, 128], F8, tag="stw",
                                  name="stw")
                nc.sync.dma_start(stw[:, 0:n, :],
                                  T_dram[kind][:, k0:k0 + n, :])
                sw = S_t[kind][:, k0:k0 + n, :]
                if build_S:
                    nc.sync.dma_start(sw, S_dram[kind][:, k0:k0 + n, :])
                # s_dst expand for each chunk of the window
                swin = swps.tile([128, WCH, H], F32, tag="swin", name="swin")
                for wi, blk in st.win_chunks[key]:
                    nc.tensor.matmul(swin[:, wi, :], stw[:, wi, :],
                                     sdst_tile[:, blk, :],
                                     start=True, stop=True,
                                     skip_group_check=True)
                tfull = xwpool.tile([128, WCH, H], F32, tag="tfull",
                                    name="tfull")
                nc.vector.tensor_tensor(tfull[:, 0:n, :], swin[:, 0:n, :],
                                        gt[:, 0:n, scol:scol + H], OP.add)
                trl = xwpool.tile([128, WCH, H], F32, tag="trl", name="trl")
                nc.vector.scalar_tensor_tensor(trl[:, 0:n, :], tfull[:, 0:n, :],
                                               NEG, tfull[:, 0:n, :],
                                               OP.mult, OP.max)
                xw = xwpool.tile([128, WCH, xww], BF16, tag="xw",
                                 name=f"xw{layer}")
                nc.scalar.activation(xw[:, 0:n, xww - H:xww],
                                     trl[:, 0:n, :], AF.Exp)
                windows[key] = (gt, xw, sw)
                return windows[key]
            return get

        # ---------------- layer 1 ----------------
        xt = ins["x_tab"]
        with (
            tc.tile_pool(name="l1g", bufs=4) as gpool,
            tc.tile_pool(name="l1xw", bufs=4) as xwpool,
            tc.tile_pool(name="l1blk", bufs=4) as blkp,
            tc.tile_pool(name="ps_swin", bufs=2, space="PSUM") as swps,
            tc.tile_pool(name="ps_st", bufs=1, space="PSUM") as stps,
            tc.tile_pool(name="ps_blk", bufs=2, space="PSUM") as psb,
            tc.tile_pool(name="ps_dense", bufs=2, space="PSUM") as psd,
            tc.tile_pool(name="ps_tr", bufs=1, space="PSUM") as pst,
        ):
            windows = {}
            getw = win_setup(1, gpool, xwpool, swps, stps,
                             xt[0:cfg.SPLIT, :], xt[cfg.SPLIT:cfg.NPOS, :],
                             cfg.XROW, 28, H1, 132, SDST1, windows, True)
            for (kind, k, first, last, ks) in st.chunks:
                if first:
                    blk_ps = psb.tile([128, 132], F32, tag="blk", name="blk")
                w, wi = divmod(ks, WCH)
                gt, xw, sw = getw(kind, w)
                # xq = x (x) w  (per-head broadcast), pair-batched
                if wi % 2 == 0:
                    nwin = len(st.win_chunks[(kind, w)])
                    pn = min(2, nwin - wi)
                    xqv = xw[:, wi:wi + pn, 0:128].rearrange(
                        "p c (h q) -> p c h q", q=32)
                    inx = gt[:, wi:wi + pn, 0:32].rearrange(
                        "p c (u q) -> p c u q", u=1) \
                        .to_broadcast((128, pn, H1, 32))
                    inw = xw[:, wi:wi + pn, 128:132].rearrange(
                        "p c (h u) -> p c h u", u=1) \
                        .to_broadcast((128, pn, H1, 32))
                    nc.vector.tensor_tensor(xqv, inx, inw, OP.mult)
                nc.tensor.matmul(blk_ps[:], sw[:, wi, :],
                                 xw[:, wi, :], start=first, stop=last,
                                 skip_group_check=True)
                if not last:
                    continue
                # ---- block end: normalize, project, ELU, h2 ----
                b = k
                dn = blkp.tile([128, H1], F32, tag="dn", name="dn")
                nc.vector.tensor_scalar(dn[:], blk_ps[:, 128:132], EPS, None,
                                        OP.add)
                rec = blkp.tile([128, H1], F32, tag="rec", name="rec")
                nc.vector.reciprocal(rec[:], dn[:])
                aggn = blkp.tile([128, 128], BF16, tag="aggn", name="aggn")
                nc.vector.tensor_tensor(
                    aggn[:].rearrange("p (h q) -> p h q", q=32),
                    blk_ps[:, 0:128].rearrange("p (h q) -> p h q", q=32),
                    rec[:].rearrange("p (h u) -> p h u", u=1)
                        .to_broadcast((128, H1, 32)),
                    OP.mult)
                tr1 = pst.tile([128, 128], BF16, tag="tr1", name="tr1")
                nc.tensor.transpose(tr1[:, 0:128], aggn[:], IDENTB[:])
                aggnT = blkp.tile([128, 128], BF16, tag="aggnT", name="aggnT")
                nc.scalar.activation(aggnT[:], tr1[:, 0:128], AF.Copy)
                out1 = psd.tile([128, 256], F32, tag="dense", name="out1")
                nc.tensor.matmul(out1[:], aggnT[:], WBIG[:], start=True,
                                 stop=True, skip_group_check=True)
                if st.add_b1:
                    nc.vector.tensor_tensor(out1[:], out1[:], B1R[:], OP.add)
                # ELU -> x2 (bf16)
                tm = blkp.tile([128, F1], BF16, tag="tm", name="tm")
                nc.scalar.activation(tm[:], out1[:], AF.Relu)
                tn = blkp.tile([128, F1], BF16, tag="tn", name="tn")
                nc.scalar.activation(tn[:], out1[:], AF.Relu, scale=-1.0)
                te = blkp.tile([128, F1], BF16, tag="te", name="te")
                nc.scalar.activation(te[:], tn[:], AF.Exp, scale=-1.0)
                x2b = blkp.tile([128, F1], BF16, tag="x2b", name="x2b")
                nc.vector.scalar_tensor_tensor(x2b[:], te[:], -1.0,
                                               tm[:], OP.add, OP.add)
                # dense layer-2 features
                tr2 = pst.tile([128, 256], BF16, tag="tr2", name="tr2")
                for q in range(2):
                    nc.tensor.transpose(tr2[:, q * 128:(q + 1) * 128],
                                        x2b[:, q * 128:(q + 1) * 128],
                                        IDENTB[:])
                x2T = blkp.tile([128, 2, 128], BF16, tag="x2T", name="x2T")
                if b >= 14:
                    nc.vector.tensor_copy(
                        x2T[:], tr2[:].rearrange("p (c j) -> p c j", j=128))
                else:
                    nc.scalar.activation(
                        x2T[:], tr2[:].rearrange("p (c j) -> p c j", j=128),
                        AF.Copy)
                h2 = psd.tile([128, 256], F32, tag="dense", name="h2")
                nc.tensor.matmul(h2[:, 0:132], x2T[:, 0, :], W2E[:, 0, :],
                                 start=True, stop=False, skip_group_check=True)
                nc.tensor.matmul(h2[:, 0:132], x2T[:, 1, :], W2E[:, 1, :],
                                 start=False, stop=True, skip_group_check=True)
                ccs = blkp.tile([128, 132], BF16, tag="ccs", name="ccs")
                if b >= 14:
                    nc.vector.tensor_copy(ccs[:], h2[:, 0:132])
                else:
                    nc.scalar.activation(ccs[:], h2[:, 0:132], AF.Copy)
                nc.sync.dma_start(cc_in[b * 128:(b + 1) * 128, 0:132], ccs[:])

        if cfg.timing_single_core:
            nc.sync.dma_start(cc_out[0:NBLK * 128, :], cc_in[:])
        else:
            nc.gpsimd.collective_compute(
                "AllGather", OP.bypass,
                replica_groups=[list(range(cfg.NC))],
                ins=[cc_in[:]],
                outs=[cc_out[:]],
            )

        # ---------------- layer 2 ----------------
        with (
            tc.tile_pool(name="l2g", bufs=3) as gpool,
            tc.tile_pool(name="l2xw", bufs=3) as xwpool,
            tc.tile_pool(name="l2blk", bufs=4) as blkp,
            tc.tile_pool(name="ps_swin2", bufs=2, space="PSUM") as swps,
            tc.tile_pool(name="ps_st2", bufs=1, space="PSUM") as stps,
            tc.tile_pool(name="ps_blk2", bufs=2, space="PSUM") as psb,
            tc.tile_pool(name="ps_tr2", bufs=1, space="PSUM") as pst,
        ):
            # own-shard s_dst2 from cc_in: [128, NBLK, 2]
            SDST2 = residp.tile([128, NBLK, H2], BF16, name="SDST2")
            nc.sync.dma_start(
                SDST2[:],
                cc_in.rearrange("(k p) c -> p k c", p=128)[:, :, 130:132])
            windows = {}
            getw = win_setup(2, gpool, xwpool, swps, stps,
                             cc_out[0:cfg.SPLIT, :], cc_out[cfg.SPLIT:cfg.NPOS, :],
                             cfg.CROW, 128, H2, 130, SDST2, windows, False)
            for (kind, k, first, last, ks) in st.chunks:
                if first:
                    blk_ps = psb.tile([128, 132], F32, tag="blk", name="blk2")
                w, wi = divmod(ks, WCH)
                gt, xw, sw = getw(kind, w)
                if wi % 2 == 0:
                    nwin = len(st.win_chunks[(kind, w)])
                    pn = min(2, nwin - wi)
                    gwv = xw[:, wi:wi + pn, 0:128].rearrange(
                        "p c (h q) -> p c h q", q=64)
                    inh = gt[:, wi:wi + pn, 0:128].rearrange(
                        "p c (h q) -> p c h q", q=64)
                    inw = xw[:, wi:wi + pn, 128:130].rearrange(
                        "p c (h u) -> p c h u", u=1) \
                        .to_broadcast((128, pn, H2, 64))
                    nc.vector.tensor_tensor(gwv, inh, inw, OP.mult)
                nc.tensor.matmul(blk_ps[:, 0:130], sw[:, wi, :],
                                 xw[:, wi, 0:130], start=first, stop=last,
                                 skip_group_check=True)
                if not last:
                    continue
                b = k
                dn = blkp.tile([128, H2], F32, tag="dn", name="dn2")
                nc.vector.tensor_scalar(dn[:], blk_ps[:, 128:130], EPS, None,
                                        OP.add)
                rec = blkp.tile([128, H2], F32, tag="rec", name="rec2")
                nc.vector.reciprocal(rec[:], dn[:])
                aggn = blkp.tile([128, 128], BF16, tag="aggn", name="aggn2")
                nc.vector.tensor_tensor(
                    aggn[:].rearrange("p (h q) -> p h q", q=64),
                    blk_ps[:, 0:128].rearrange("p (h q) -> p h q", q=64),
                    rec[:].rearrange("p (h u) -> p h u", u=1)
                        .to_broadcast((128, H2, 64)),
                    OP.mult)
                if st.add_b2:
                    nc.vector.tensor_tensor(aggn[:], aggn[:], B2R[:], OP.add)
                tm = blkp.tile([128, F2], BF16, tag="tm", name="tm2")
                nc.scalar.activation(tm[:], aggn[:], AF.Relu)
                tn = blkp.tile([128, F2], BF16, tag="tn", name="tn2")
                nc.scalar.activation(tn[:], aggn[:], AF.Relu, scale=-1.0)
                te = blkp.tile([128, F2], BF16, tag="te", name="te2")
                nc.scalar.activation(te[:], tn[:], AF.Exp, scale=-1.0)
                x3 = blkp.tile([128, F2], BF16, tag="x3", name="x3")
                nc.vector.scalar_tensor_tensor(x3[:], te[:], -1.0, tm[:],
                                               OP.add, OP.add)
                tr3 = pst.tile([128, 128], BF16, tag="tr", name="tr3")
                nc.tensor.transpose(tr3[:], x3[:], IDENTB[:])
                x3T = blkp.tile([128, 128], BF16, tag="x3T", name="x3T")
                nc.scalar.activation(x3T[:], tr3[:], AF.Copy)
                zp = pst.tile([128, 2], F32, tag="zp", name="zp")
                nc.tensor.matmul(zp[:, 0:1], x3T[:], WFC[:],
                                 start=True, stop=True, skip_group_check=True)
                nc.scalar.activation(z_all[:, b:b + 1], zp[:, 0:1], AF.Copy)

        # final sigmoid + output
        with tc.tile_pool(name="fin", bufs=1) as finp:
            ys = finp.tile([128, NBLK], F32, name="ys")
            bfc = float(np.asarray(st.bfc).reshape(-1)[0])
            nc.scalar.activation(ys[:], z_all[:], AF.Sigmoid, bias=bfc)
            nc.sync.dma_start(y[:, :], ys[:])


# --------------------------------------------------------------------------
#  host entry
# --------------------------------------------------------------------------

def build(inputs, cfg: Cfg):
    ei = np.asarray(inputs["edge_index"])
    loops = np.arange(cfg.N, dtype=ei.dtype)
    src = np.concatenate([ei[0], loops])
    dst = np.concatenate([ei[1], loops])
    st = prep_edges(cfg, src, dst)
    st.add_b1 = bool(np.any(np.asarray(inputs["b1"])))
    st.add_b2 = bool(np.any(np.asarray(inputs["b2"])))
    st.bfc = np.asarray(inputs["bfc"], np.float32)
    in_maps = host_inputs(cfg, st, inputs)

    nc = bacc.Bacc("TRN2", target_bir_lowering=False, debug=False,
                   num_devices=cfg.NC, dynamic_dma_scratch_size=65536)
    ins_aps = {}
    for k, v in in_maps[0].items():
        dt = mybir.dt.from_np(v.dtype)
        ins_aps[k] = nc.dram_tensor(k, list(v.shape), dt,
                                    kind="ExternalInput").ap()
    y_ap = nc.dram_tensor("y", [128, cfg.NBLK], F32, kind="ExternalOutput").ap()

    with tile.TileContext(nc) as tc:
        emit_gat(tc, {"y": y_ap}, ins_aps, cfg, st)
    nc.compile()
    return nc, in_maps, st


def build_and_run(inputs, cfg: Cfg, trace=False):
    nc, in_maps, st = build(inputs, cfg)
    res = run_bass_kernel_spmd(nc, in_maps, core_ids=list(range(cfg.NC)),
                               trace=trace)
    out = np.zeros((cfg.N, 1), np.float32)
    for c in range(cfg.NC):
        yc = res.results[c]["y"]          # [128, NBLK]
        for k, b in enumerate(st.deal[c]):
            rows = min(128, cfg.N - b * 128)
            out[b * 128:b * 128 + rows, 0] = yc[:rows, k]
    return out, res


def kernel(**inputs):
    cfg = Cfg()
    out, _ = build_and_run(inputs, cfg)
    return out.astype(np.float32)
